# revision 1
# baseline (speedup 1.0000x reference)
"""Trainium2 Bass kernel for a top-2 MoE classifier head (B=4096, D=1024,
E=16 experts, H=2048, C=1000 classes, k=2), expert-parallel over 8 NeuronCores.

Strategy (per core i of 8):
  - inputs: full x (replicated), gate weights (replicated), this core's 2
    experts' W1/b1/W2/b2 slices, a one-hot "which experts are mine" selector.
  - on device: transpose x tile-by-tile on the PE (fp32 has no DMA-transpose),
    compute gate logits + softmax + top-2 masks for ALL tokens (routing is
    replicated, so no token exchange is needed), stream-compact the tokens
    routed to the 2 local experts via prefix-scan + one indirect-DMA scatter
    of small (token-id, wv0, wv1) metadata rows, gather the selected x rows,
    run the expert FFN (fc1 -> fc2), log_softmax over classes, weight by the
    gate prob, and scatter-add the weighted rows into a [4096,1000] partial.
  - ReduceScatter(sum) across the 8 cores leaves each core with the final
    [512,1000] slice of the output; the host just concatenates the 8 slices.
"""

import numpy as np

import bass_rust
import concourse.bass as bass
import concourse.mybir as mybir
import concourse.tile as tile
from concourse.bass_utils import run_bass_kernel_spmd
from concourse.masks import make_identity
from concourse.vector_clock import ScopedClock

# ---------------------------------------------------------------- constants
B, D, E, H, C, TOPK = 4096, 1024, 16, 2048, 1000, 2
N_CORES = 8
P = 128
NT = B // P            # 32 token tiles
DC = D // P            # 8 d-chunks
HC = H // P            # 16 h-chunks
ELOC = E // N_CORES    # 2 experts per core
CAP = 640              # capacity (slots) per expert; true load ~512 +- 22
CAPT = CAP // P        # 5 slot tiles per expert
NSLOT = ELOC * CAP     # 1280 rows in the per-core dispatch list
TRASH = B              # scatter target for padding rows (row B of partial)
OOB = 1 << 20          # sentinel offset for "not my expert" (bounds-checked away)
FP32 = mybir.dt.float32
INT32 = mybir.dt.int32

_COMBINE = "rs"        # "rs" = on-device ReduceScatter; "hostsum" = host combine


# ------------------------------------------------- walrus compatibility patch
# This neuronxcc/walrus build rejects sync waits on CTRL-class instructions
# (Drain/NoOp) beyond a tiny cap ("Too many sync wait commands").  Tile's exit
# sequence puts the global-clock waits on the final drain and an eq-wait plus
# update on each barrier drain.  Hoist every wait onto its own single-wait NoOp
# immediately before the owning instruction; semantics are unchanged (same
# engine, program order).
def _install_walrus_compat():
    if getattr(bass.Bass, "_moe_walrus_patched", False):
        return

    def _patched_meb(self, engines):
        for inst in self._multi_engine_barrier_insts(engines):
            eng = self.engines[inst.engine]
            si = inst.sync_info
            if si is not None and si.on_wait:
                for w in list(si.on_wait):
                    nop = eng.nop(nofuse=True)
                    nop.ins.sync_info = bass_rust.SyncInfo(on_wait=[w], on_update=[])
                inst.sync_info = bass_rust.SyncInfo(
                    on_wait=[], on_update=list(si.on_update)
                )
            eng.add_instruction(inst)

    bass.Bass.multi_engine_barrier = _patched_meb

    def _patched_dab(self, tick_clock, wait_clock):
        drain_inst = self.nc.sync.drain()
        wait_clock.add_sem_waits(
            drain_inst.ins, ScopedClock({None: tick_clock.global_clock})
        )
        si = drain_inst.ins.sync_info
        if si is not None and si.on_wait:
            waits = list(si.on_wait)
            drain_inst.ins.sync_info = bass_rust.SyncInfo(
                on_wait=[], on_update=list(si.on_update)
            )
            for w in waits:
                nop = self.nc.sync.nop(nofuse=True)
                nop.ins.sync_info = bass_rust.SyncInfo(on_wait=[w], on_update=[])
            self.nc.sync.drain()
        self.nc.all_engine_barrier()
        popped = self.nc._tile_sem_poison_stack.pop()
        assert popped is self._sem_poison
        self.nc.clear_and_free_semaphores(list(self.sems.allocated().values()))
        self.nc.all_engine_barrier()

    tile.TileContext._drain_and_barrier = _patched_dab
    bass.Bass._moe_walrus_patched = True


_waitnop_ctr = [0]


def _split_excess_waits(nc):
    """Walrus compat, part 2: this build also rejects >1 sync wait on ordinary
    instructions (DMA pseudo-ops included).  Rewrite every block so each
    instruction carries at most one wait (none on Drain); excess waits move to
    single-wait NoOps inserted immediately before, on the same engine."""
    for bb in nc.main_func.blocks:
        new = []
        changed = False
        for inst in bb.instructions:
            si = inst.sync_info
            if si is not None and si.on_wait:
                waits = list(si.on_wait)
                keep_budget = 0 if type(inst).__name__ == "InstDrain" else 1
                if len(waits) > keep_budget:
                    keep = waits[:keep_budget]
                    for w in waits[keep_budget:]:
                        _waitnop_ctr[0] += 1
                        nop = mybir.InstNoOp(
                            name=f"I-waitnop-{_waitnop_ctr[0]}", ins=[], outs=[]
                        )
                        nop.engine = inst.engine
                        nop.sync_info = bass_rust.SyncInfo(
                            on_wait=[w], on_update=[]
                        )
                        new.append(nop)
                    inst.sync_info = bass_rust.SyncInfo(
                        on_wait=keep, on_update=list(si.on_update)
                    )
                    changed = True
            new.append(inst)
        if changed:
            bb.instructions = new


# ------------------------------------------------------------- kernel builder
def build_bass(combine: str = _COMBINE, repeat: int = 1, for_sim: bool = False) -> bass.Bass:
    _install_walrus_compat()
    nc = bass.Bass()

    x_in = nc.dram_tensor("x", [B, D], FP32, kind="ExternalInput")
    wg_in = nc.dram_tensor("wg", [D, E], FP32, kind="ExternalInput")
    bg_in = nc.dram_tensor("bg", [P, E], FP32, kind="ExternalInput")
    w1_in = nc.dram_tensor("w1", [ELOC, D, H], FP32, kind="ExternalInput")
    b1_in = nc.dram_tensor("b1", [ELOC, H], FP32, kind="ExternalInput")
    w2_in = nc.dram_tensor("w2", [ELOC, H, C], FP32, kind="ExternalInput")
    b2_in = nc.dram_tensor("b2", [ELOC, P, C], FP32, kind="ExternalInput")
    sel_in = nc.dram_tensor("sel", [ELOC, P, E], FP32, kind="ExternalInput")
    if combine == "rs":
        out_t = nc.dram_tensor("out", [B // N_CORES, C], FP32, kind="ExternalOutput")
    else:
        out_t = nc.dram_tensor("out", [B, C], FP32, kind="ExternalOutput")

    with tile.TileContext(nc) as tc:
        for _ in range(repeat):
            _emit_body(
                nc, tc, x_in, wg_in, bg_in, w1_in, b1_in, w2_in, b2_in, sel_in,
                out_t, combine,
            )
    if not for_sim:
        _split_excess_waits(nc)
    return nc


def _emit_body(nc, tc, x_in, wg_in, bg_in, w1_in, b1_in, w2_in, b2_in, sel_in,
               out_t, combine):
    from contextlib import ExitStack

    with ExitStack() as ctx:
        dram = ctx.enter_context(tc.tile_pool(name="dram", bufs=1, space="DRAM"))
        consts = ctx.enter_context(tc.tile_pool(name="consts", bufs=1))
        xrow_p = ctx.enter_context(tc.tile_pool(name="xrow", bufs=3))
        xt_p = ctx.enter_context(tc.tile_pool(name="xt", bufs=3))
        gate_p = ctx.enter_context(tc.tile_pool(name="gate", bufs=4))
        route_p = ctx.enter_context(tc.tile_pool(name="route", bufs=1))
        wres_p = ctx.enter_context(tc.tile_pool(name="wres", bufs=1))
        ht_p = ctx.enter_context(tc.tile_pool(name="ht", bufs=1))
        xdt_p = ctx.enter_context(tc.tile_pool(name="xdt", bufs=1))
        ffn_p = ctx.enter_context(tc.tile_pool(name="ffn", bufs=2))
        psum = ctx.enter_context(tc.tile_pool(name="psum", bufs=2, space="PSUM"))
        psg = ctx.enter_context(tc.tile_pool(name="psg", bufs=1, space="PSUM"))

        # ------------------------------------------------ DRAM scratch
        idlist = dram.tile([NSLOT, 4], FP32)
        partial = dram.tile([B + P, C], FP32)
        if combine == "rs":
            rsout = dram.tile([B // N_CORES, C], FP32)

        # ------------------------------------------------ constants
        ident = consts.tile([P, P], FP32)
        make_identity(nc, ident[:])

        # strict upper-triangular ones (TRI[p, c] = 1 iff c > p) for the
        # cross-partition exclusive prefix sum
        tri_i = consts.tile([P, P], INT32)
        nc.gpsimd.iota(tri_i[:], pattern=[[1, P]], base=0, channel_multiplier=-1)
        tri = consts.tile([P, P], FP32)
        nc.vector.tensor_scalar(tri[:], tri_i[:], 0, None, op0=mybir.AluOpType.is_gt)

        wgsb = consts.tile([P, DC, E], FP32)
        nc.sync.dma_start(wgsb[:], wg_in[:].rearrange("(c p) e -> p c e", p=P))
        bgbc = consts.tile([P, E], FP32)
        nc.sync.dma_start(bgbc[:], bg_in[:])
        selbc = consts.tile([P, ELOC, E], FP32)
        nc.sync.dma_start(selbc[:], sel_in[:].rearrange("j p e -> p j e"))
        b1sb = consts.tile([P, ELOC, HC], FP32)
        nc.sync.dma_start(
            b1sb[:], b1_in[:].rearrange("j (m p) -> p j m", p=P)
        )
        b2bc = consts.tile([P, ELOC, C], FP32)
        nc.sync.dma_start(b2bc[:], b2_in[:].rearrange("j p c -> p j c"))

        # token ids, f32 (exact for < 2^24)
        tokid_i = consts.tile([P, NT], INT32)
        nc.gpsimd.iota(tokid_i[:], pattern=[[P, NT]], base=0, channel_multiplier=1)
        tokf = consts.tile([P, NT], FP32)
        nc.vector.tensor_copy(tokf[:], tokid_i[:])

        # ------------------------------------------------ zero-init partial
        zsb = consts.tile([P, C], FP32)
        nc.vector.memset(zsb[:], 0.0)
        for i in range((B + P) // P):
            nc.sync.dma_start(partial[i * P:(i + 1) * P, :], zsb[:])

        # prefill idlist with (id=TRASH, wv=0, wv=0, 0) for padding slots
        pref = consts.tile([P, NSLOT // P, 4], FP32)
        nc.vector.memset(pref[:], 0.0)
        nc.vector.memset(pref[:, :, 3:4], float(TRASH))
        nc.sync.dma_start(
            idlist[:].rearrange("(a p) m -> p a m", p=P), pref[:]
        )

        # ------------------------------------------------ routing buffers
        g0buf = route_p.tile([P, NT], FP32)    # gate prob of local expert 0
        g1buf = route_p.tile([P, NT], FP32)
        mU = [route_p.tile([P, NT], FP32, name=f"mU{j}") for j in range(ELOC)]
        m1b = [route_p.tile([P, NT], FP32, name=f"m1b{j}") for j in range(ELOC)]

        gbufs = [g0buf, g1buf]

        # ================================================ phase A: gate+masks
        for f in range(NT):
            xrow = xrow_p.tile([P, D], FP32, tag="xrow")
            nc.sync.dma_start(xrow[:], x_in[f * P:(f + 1) * P, :])

            xt = xt_p.tile([P, DC, P], FP32, tag="xt")
            for c in range(DC):
                pst = psum.tile([P, P], FP32, tag="pst")
                nc.tensor.matmul(
                    pst[:], lhsT=xrow[:, c * P:(c + 1) * P], rhs=ident[:],
                    start=True, stop=True,
                )
                nc.vector.tensor_copy(xt[:, c, :], pst[:])

            gps = psg.tile([P, E], FP32, tag="gps")
            for c in range(DC):
                nc.tensor.matmul(
                    gps[:], lhsT=xt[:, c, :], rhs=wgsb[:, c, :],
                    start=(c == 0), stop=(c == DC - 1),
                )

            glog = gate_p.tile([P, E], FP32, tag="glog")
            nc.vector.tensor_add(glog[:], gps[:], bgbc[:])
            mx8 = gate_p.tile([P, 8], FP32, tag="mx8")
            nc.vector.max(out=mx8[:], in_=glog[:])
            nm = gate_p.tile([P, 1], FP32, tag="nm")
            nc.vector.tensor_scalar_mul(nm[:], mx8[:, 0:1], -1.0)
            exps = gate_p.tile([P, E], FP32, tag="exps")
            se = gate_p.tile([P, 1], FP32, tag="se")
            nc.scalar.activation(
                exps[:], glog[:], mybir.ActivationFunctionType.Exp,
                bias=nm[:], scale=1.0, accum_out=se[:],
            )
            rs = gate_p.tile([P, 1], FP32, tag="rs")
            nc.vector.reciprocal(rs[:], se[:])
            gate = gate_p.tile([P, E], FP32, tag="gateprob")
            nc.vector.tensor_scalar_mul(gate[:], exps[:], rs[:])

            m8g = gate_p.tile([P, 8], FP32, tag="m8g")
            nc.vector.max(out=m8g[:], in_=gate[:])

            scr = gate_p.tile([P, E], FP32, tag="scr")
            for j in range(ELOC):
                gs = gbufs[j][:, f:f + 1]
                nc.vector.tensor_mul(scr[:], gate[:], selbc[:, j, :])
                nc.vector.reduce_sum(gs, scr[:], axis=mybir.AxisListType.X)
                nc.vector.tensor_tensor(
                    out=mU[j][:, f:f + 1], in0=gs, in1=m8g[:, 1:2],
                    op=mybir.AluOpType.is_ge,
                )
                nc.vector.tensor_tensor(
                    out=m1b[j][:, f:f + 1], in0=gs, in1=m8g[:, 0:1],
                    op=mybir.AluOpType.is_ge,
                )

        # ================================================ phase B: compaction
        # per-partition inclusive prefix along f, then exclusive via subtract
        rank = []
        tot = route_p.tile([P, ELOC], FP32)
        for j in range(ELOC):
            inc = route_p.tile([P, NT], FP32, tag=f"inc{j}")
            nc.vector.tensor_tensor_scan(
                out=inc[:], data0=mU[j][:], data1=mU[j][:], initial=0.0,
                op0=mybir.AluOpType.add, op1=mybir.AluOpType.bypass,
            )
            exc = route_p.tile([P, NT], FP32, tag=f"exc{j}")
            nc.vector.tensor_sub(exc[:], inc[:], mU[j][:])
            nc.vector.tensor_copy(tot[:, j:j + 1], inc[:, NT - 1:NT])
            rank.append(exc)

        offp = psg.tile([P, ELOC], FP32, tag="offp")
        nc.tensor.matmul(offp[:], lhsT=tri[:], rhs=tot[:], start=True, stop=True)
        offs = route_p.tile([P, ELOC], FP32)
        nc.vector.tensor_copy(offs[:], offp[:])

        for j in range(ELOC):
            nc.vector.tensor_scalar(
                rank[j][:], rank[j][:], offs[:, j:j + 1], float(j * CAP),
                op0=mybir.AluOpType.add, op1=mybir.AluOpType.add,
            )

        # slots per choice; non-local tokens get OOB
        slots_i = []
        for choice in range(2):
            own = route_p.tile([P, NT], FP32, tag=f"own{choice}")
            slot = route_p.tile([P, NT], FP32, tag=f"slot{choice}")
            nc.vector.memset(slot[:], 0.0)
            nc.vector.memset(own[:], 0.0)
            for j in range(ELOC):
                cm = route_p.tile([P, NT], FP32, tag="cm")
                if choice == 0:
                    nc.vector.tensor_copy(cm[:], m1b[j][:])
                else:
                    nc.vector.tensor_sub(cm[:], mU[j][:], m1b[j][:])
                nc.vector.tensor_add(own[:], own[:], cm[:])
                nc.vector.tensor_mul(cm[:], cm[:], rank[j][:])
                nc.vector.tensor_add(slot[:], slot[:], cm[:])
            # slot += (1 - own) * OOB
            nc.vector.tensor_scalar(
                own[:], own[:], -float(OOB), float(OOB),
                op0=mybir.AluOpType.mult, op1=mybir.AluOpType.add,
            )
            nc.vector.tensor_add(slot[:], slot[:], own[:])
            si = route_p.tile([P, NT], INT32, tag=f"sloti{choice}")
            nc.vector.tensor_copy(si[:], slot[:])
            slots_i.append(si)

        # metadata rows (tokid, gsel0, gsel1, 0) scattered into idlist
        meta = route_p.tile([P, NT, 4], FP32)
        nc.vector.tensor_copy(meta[:, :, 0], tokf[:])
        nc.vector.tensor_copy(meta[:, :, 1], g0buf[:])
        nc.vector.tensor_copy(meta[:, :, 2], g1buf[:])
        nc.vector.tensor_copy(meta[:, :, 3], tokf[:])
        # HW only honors [128, 1] offset vectors (multi-column offsets are a
        # sim-only behavior), so emit one small scatter per (choice, token tile)
        bc_reg = nc.gpsimd.to_reg(NSLOT - 1)
        for choice in range(2):
            for f in range(NT):
                nc.gpsimd.indirect_dma_start(
                    out=idlist[:],
                    out_offset=bass.IndirectOffsetOnAxis(
                        ap=slots_i[choice][:, f:f + 1], axis=0
                    ),
                    in_=meta[:, f, :],
                    in_offset=None,
                    bounds_check=bc_reg,
                    oob_is_err=False,
                )

        # read the compacted list back
        idsb = route_p.tile([P, NSLOT // P, 4], FP32)
        nc.sync.dma_start(idsb[:], idlist[:].rearrange("(a p) m -> p a m", p=P))
        idcast = route_p.tile([P, NSLOT // P], INT32)
        nc.vector.tensor_copy(idcast[:], idsb[:, :, 0])
        idscat = route_p.tile([P, NSLOT // P], INT32)
        nc.vector.tensor_copy(idscat[:], idsb[:, :, 3])

        # ================================================ phase C: expert FFN
        for j in range(ELOC):
            # gather this expert's x rows tile by tile: xg[p, :] = x[id, :],
            # then transpose -> xdt[:, c, slot]
            xdt = xdt_p.tile([P, DC, CAP], FP32, tag="xdt")
            for a in range(CAPT):
                xg = ffn_p.tile([P, D], FP32, tag="xg")
                nc.gpsimd.indirect_dma_start(
                    out=xg[:],
                    out_offset=None,
                    in_=x_in[:],
                    in_offset=bass.IndirectOffsetOnAxis(
                        ap=idcast[:, j * CAPT + a:j * CAPT + a + 1], axis=0
                    ),
                )
                for c in range(DC):
                    pst = psum.tile([P, P], FP32, tag="pst")
                    nc.tensor.matmul(
                        pst[:], lhsT=xg[:, c * P:(c + 1) * P], rhs=ident[:],
                        start=True, stop=True,
                    )
                    nc.vector.tensor_copy(xdt[:, c, a * P:(a + 1) * P], pst[:])

            # fc1: hT[m-tile] = sum_c W1[c,m]^T @ xdt[c]  (+ b1)
            w1r = wres_p.tile([P, DC, H], FP32, tag="wres")
            nc.sync.dma_start(
                w1r[:], w1_in[j].rearrange("(c p) h -> p c h", p=P)
            )
            hts = ht_p.tile([P, HC, CAP], FP32, tag="hts")
            segs = [(0, 512), (512, CAP)]
            for m in range(HC):
                for s0, s1 in segs:
                    hp = psum.tile([P, 512], FP32, tag="hp")
                    for c in range(DC):
                        nc.tensor.matmul(
                            hp[:, :s1 - s0],
                            lhsT=w1r[:, c, m * P:(m + 1) * P],
                            rhs=xdt[:, c, s0:s1],
                            start=(c == 0), stop=(c == DC - 1),
                        )
                    nc.vector.tensor_scalar(
                        hts[:, m, s0:s1], hp[:, :s1 - s0],
                        b1sb[:, j, m:m + 1], None, op0=mybir.AluOpType.add,
                    )

            # fc2 + log_softmax + weight + scatter-add.  W2 reuses the W1
            # residency slot (same tag) — their lifetimes are sequential.
            w2r = wres_p.tile([P, HC, C], FP32, tag="wres")
            nc.sync.dma_start(
                w2r[:], w2_in[j].rearrange("(kc p) cc -> p kc cc", p=P)
            )
            csegs = [(0, 512), (512, C)]
            for a in range(CAPT):
                lps = []
                for s0, s1 in csegs:
                    lp = psum.tile([P, 512], FP32, tag="lp")
                    for kc in range(HC):
                        nc.tensor.matmul(
                            lp[:, :s1 - s0],
                            lhsT=hts[:, kc, a * P:(a + 1) * P],
                            rhs=w2r[:, kc, s0:s1],
                            start=(kc == 0), stop=(kc == HC - 1),
                        )
                    lps.append(lp)
                pst_sb = ffn_p.tile([P, C], FP32, tag="logits")
                for (s0, s1), lp in zip(csegs, lps):
                    nc.vector.tensor_add(
                        pst_sb[:, s0:s1], lp[:, :s1 - s0], b2bc[:, j, s0:s1]
                    )
                mx = ffn_p.tile([P, 1], FP32, tag="mx")
                nc.vector.reduce_max(mx[:], pst_sb[:], axis=mybir.AxisListType.X)
                nmx = ffn_p.tile([P, 1], FP32, tag="nmx")
                nc.vector.tensor_scalar_mul(nmx[:], mx[:], -1.0)
                expv = ffn_p.tile([P, C], FP32, tag="expv")
                sev = ffn_p.tile([P, 1], FP32, tag="sev")
                nc.scalar.activation(
                    expv[:], pst_sb[:], mybir.ActivationFunctionType.Exp,
                    bias=nmx[:], scale=1.0, accum_out=sev[:],
                )
                lnz = ffn_p.tile([P, 1], FP32, tag="lnz")
                nc.scalar.activation(
                    lnz[:], sev[:], mybir.ActivationFunctionType.Ln,
                )
                total = ffn_p.tile([P, 1], FP32, tag="total")
                nc.vector.tensor_add(total[:], mx[:], lnz[:])
                outsb = ffn_p.tile([P, C], FP32, tag="outsb")
                nc.vector.tensor_scalar(
                    outsb[:], pst_sb[:], total[:],
                    idsb[:, j * CAPT + a, 1 + j:2 + j],
                    op0=mybir.AluOpType.subtract, op1=mybir.AluOpType.mult,
                )
                # expert 0 scatters onto freshly zeroed rows: plain writes
                # (pad rows all write zeros to the trash row - benign); expert 1
                # must accumulate (a token can route to both local experts).
                nc.gpsimd.indirect_dma_start(
                    out=partial[:],
                    out_offset=bass.IndirectOffsetOnAxis(
                        ap=idscat[:, j * CAPT + a:j * CAPT + a + 1], axis=0
                    ),
                    in_=outsb[:],
                    in_offset=None,
                    compute_op=(mybir.AluOpType.bypass if j == 0
                                else mybir.AluOpType.add),
                )

        # ================================================ phase D: combine
        if combine == "rs":
            nc.gpsimd.collective_compute(
                "ReduceScatter",
                mybir.AluOpType.add,
                replica_groups=[list(range(N_CORES))],
                ins=[partial[:B, :].opt()],
                outs=[rsout[:].opt()],
            )
            ot = ffn_p.tile([P, C], FP32, tag="otile")
            for i in range(B // N_CORES // P):
                nc.sync.dma_start(ot[:], rsout[i * P:(i + 1) * P, :])
                nc.sync.dma_start(out_t[i * P:(i + 1) * P, :], ot[:])
        else:
            ot = ffn_p.tile([P, C], FP32, tag="otile")
            for i in range(B // P):
                nc.sync.dma_start(ot[:], partial[i * P:(i + 1) * P, :])
                nc.sync.dma_start(out_t[i * P:(i + 1) * P, :], ot[:])


# ---------------------------------------------------------------- host glue
_CACHE = {}


def _get_nc(combine: str):
    if combine not in _CACHE:
        _CACHE[combine] = build_bass(combine)
    return _CACHE[combine]


def make_in_maps(x, Wg, bg, W1, b1, W2, b2):
    x = np.ascontiguousarray(np.asarray(x, np.float32))
    Wg = np.ascontiguousarray(np.asarray(Wg, np.float32))
    bg = np.tile(np.asarray(bg, np.float32).reshape(1, E), (P, 1))
    W1 = np.ascontiguousarray(np.asarray(W1, np.float32))
    b1 = np.ascontiguousarray(np.asarray(b1, np.float32))
    W2 = np.ascontiguousarray(np.asarray(W2, np.float32))
    b2 = np.ascontiguousarray(np.asarray(b2, np.float32))
    maps = []
    for i in range(N_CORES):
        lo = i * ELOC
        sel = np.zeros((ELOC, 1, E), np.float32)
        for j in range(ELOC):
            sel[j, 0, lo + j] = 1.0
        # replicated across partitions host-side (partition_broadcast is
        # unsupported by this walrus build)
        maps.append({
            "x": x,
            "wg": Wg,
            "bg": bg,
            "w1": np.ascontiguousarray(W1[lo:lo + ELOC]),
            "b1": np.ascontiguousarray(b1[lo:lo + ELOC]),
            "w2": np.ascontiguousarray(W2[lo:lo + ELOC]),
            "b2": np.ascontiguousarray(
                np.tile(b2[lo:lo + ELOC].reshape(ELOC, 1, C), (1, P, 1))),
            "sel": np.ascontiguousarray(np.tile(sel, (1, P, 1))),
        })
    return maps


def _assert_capacity(x, Wg, bg):
    gate = np.asarray(x, np.float32) @ np.asarray(Wg, np.float32)
    gate += np.asarray(bg, np.float32).reshape(1, E)
    order = np.argsort(-gate, axis=1)[:, :TOPK]
    counts = np.bincount(order.ravel(), minlength=E)
    assert counts.max() <= CAP, (
        f"per-expert token load {counts.max()} exceeds CAP={CAP}; "
        f"raise CAP in kernel.py"
    )


def kernel(x, Wg, bg, W1, b1, W2, b2, k):
    assert int(k) == TOPK
    _assert_capacity(x, Wg, bg)
    combine = _COMBINE
    nc = _get_nc(combine)
    maps = make_in_maps(x, Wg, bg, W1, b1, W2, b2)
    res = run_bass_kernel_spmd(nc, maps, core_ids=list(range(N_CORES)))
    if combine == "rs":
        out = np.concatenate([res.results[i]["out"] for i in range(N_CORES)], axis=0)
    else:
        out = np.sum([res.results[i]["out"] for i in range(N_CORES)], axis=0)
    return out.astype(np.float32)


if __name__ == "__main__":
    rng = np.random.default_rng(0)
    x = rng.standard_normal((B, D), np.float32)
    Wg = rng.standard_normal((D, E), np.float32) / np.sqrt(D)
    bg = np.zeros((E,), np.float32)
    W1 = (rng.standard_normal((E, D, H)) / np.sqrt(D)).astype(np.float32)
    b1 = np.zeros((E, H), np.float32)
    W2 = (rng.standard_normal((E, H, C)) / np.sqrt(H)).astype(np.float32)
    b2 = np.zeros((E, C), np.float32)
    out = kernel(x, Wg, bg, W1, b1, W2, b2, 2)
    print("kernel ran, out:", out.shape, out.dtype, float(np.abs(out).max()))



# revision 21
# speedup vs baseline: 185.1490x; 185.1490x over previous
"""Trainium2 Bass kernel for a top-2 MoE classifier head (B=4096, D=1024,
E=16 experts, H=2048, C=1000 classes, k=2), expert-parallel over 8 NeuronCores.

Strategy (per core i of 8):
  - inputs: full x (replicated), gate weights (replicated), this core's 2
    experts' W1/b1/W2/b2 slices, a one-hot "which experts are mine" selector.
  - on device: transpose x tile-by-tile on the PE (fp32 has no DMA-transpose),
    compute gate logits + softmax + top-2 masks for ALL tokens (routing is
    replicated, so no token exchange is needed), stream-compact the tokens
    routed to the 2 local experts via prefix-scan + one indirect-DMA scatter
    of small (token-id, wv0, wv1) metadata rows, gather the selected x rows,
    run the expert FFN (fc1 -> fc2), log_softmax over classes, weight by the
    gate prob, and scatter-add the weighted rows into a [4096,1000] partial.
  - ReduceScatter(sum) across the 8 cores leaves each core with the final
    [512,1000] slice of the output; the host just concatenates the 8 slices.
"""

import numpy as np

import bass_rust
import concourse.bass as bass
import concourse.mybir as mybir
import concourse.tile as tile
from concourse.bass_utils import run_bass_kernel_spmd
from concourse.masks import make_identity
from concourse.vector_clock import ScopedClock

# ---------------------------------------------------------------- constants
B, D, E, H, C, TOPK = 4096, 1024, 16, 2048, 1000, 2
N_CORES = 8
P = 128
NT = B // P            # 32 token tiles
DC = D // P            # 8 d-chunks
HC = H // P            # 16 h-chunks
ELOC = E // N_CORES    # 2 experts per core
CAP = 640              # capacity (slots) per expert; true load ~512 +- 22
CAPT = CAP // P        # 5 slot tiles per expert
NSLOT = ELOC * CAP     # 1280 rows in the per-core dispatch list
TRASH = B              # scatter target for padding rows (row B of partial)
OOB = 1 << 20          # sentinel offset for "not my expert" (bounds-checked away)
FP32 = mybir.dt.float32
BF16 = mybir.dt.bfloat16
INT32 = mybir.dt.int32

_COMBINE = "rs"        # "rs" = on-device ReduceScatter; "hostsum" = host combine


# ------------------------------------------------- walrus compatibility patch
# This neuronxcc/walrus build rejects sync waits on CTRL-class instructions
# (Drain/NoOp) beyond a tiny cap ("Too many sync wait commands").  Tile's exit
# sequence puts the global-clock waits on the final drain and an eq-wait plus
# update on each barrier drain.  Hoist every wait onto its own single-wait NoOp
# immediately before the owning instruction; semantics are unchanged (same
# engine, program order).
def _install_walrus_compat():
    if getattr(bass.Bass, "_moe_walrus_patched", False):
        return

    def _patched_meb(self, engines):
        for inst in self._multi_engine_barrier_insts(engines):
            eng = self.engines[inst.engine]
            si = inst.sync_info
            if si is not None and si.on_wait:
                for w in list(si.on_wait):
                    nop = eng.nop(nofuse=True)
                    nop.ins.sync_info = bass_rust.SyncInfo(on_wait=[w], on_update=[])
                inst.sync_info = bass_rust.SyncInfo(
                    on_wait=[], on_update=list(si.on_update)
                )
            eng.add_instruction(inst)

    bass.Bass.multi_engine_barrier = _patched_meb

    def _patched_dab(self, tick_clock, wait_clock):
        drain_inst = self.nc.sync.drain()
        wait_clock.add_sem_waits(
            drain_inst.ins, ScopedClock({None: tick_clock.global_clock})
        )
        si = drain_inst.ins.sync_info
        if si is not None and si.on_wait:
            waits = list(si.on_wait)
            drain_inst.ins.sync_info = bass_rust.SyncInfo(
                on_wait=[], on_update=list(si.on_update)
            )
            for w in waits:
                nop = self.nc.sync.nop(nofuse=True)
                nop.ins.sync_info = bass_rust.SyncInfo(on_wait=[w], on_update=[])
            self.nc.sync.drain()
        self.nc.all_engine_barrier()
        popped = self.nc._tile_sem_poison_stack.pop()
        assert popped is self._sem_poison
        self.nc.clear_and_free_semaphores(list(self.sems.allocated().values()))
        self.nc.all_engine_barrier()

    tile.TileContext._drain_and_barrier = _patched_dab
    bass.Bass._moe_walrus_patched = True


_waitnop_ctr = [0]


def _split_excess_waits(nc):
    """Walrus compat, part 2: this build also rejects >1 sync wait on ordinary
    instructions (DMA pseudo-ops included).  Rewrite every block so each
    instruction carries at most one wait (none on Drain); excess waits move to
    single-wait NoOps inserted immediately before, on the same engine."""
    for bb in nc.main_func.blocks:
        new = []
        changed = False
        for inst in bb.instructions:
            si = inst.sync_info
            if si is not None and si.on_wait:
                waits = list(si.on_wait)
                keep_budget = 0 if type(inst).__name__ == "InstDrain" else 1
                if len(waits) > keep_budget:
                    keep = waits[:keep_budget]
                    for w in waits[keep_budget:]:
                        _waitnop_ctr[0] += 1
                        nop = mybir.InstNoOp(
                            name=f"I-waitnop-{_waitnop_ctr[0]}", ins=[], outs=[]
                        )
                        nop.engine = inst.engine
                        nop.sync_info = bass_rust.SyncInfo(
                            on_wait=[w], on_update=[]
                        )
                        new.append(nop)
                    inst.sync_info = bass_rust.SyncInfo(
                        on_wait=keep, on_update=list(si.on_update)
                    )
                    changed = True
            new.append(inst)
        if changed:
            bb.instructions = new


# ------------------------------------------------------------- kernel builder
def build_bass(combine: str = _COMBINE, repeat: int = 1, for_sim: bool = False,
               noind: bool = False) -> bass.Bass:
    _install_walrus_compat()
    nc = bass.Bass()

    x_in = nc.dram_tensor("x", [B, D], FP32, kind="ExternalInput")
    wg_in = nc.dram_tensor("wg", [D, E], FP32, kind="ExternalInput")
    bg_in = nc.dram_tensor("bg", [P, E], FP32, kind="ExternalInput")
    w1_in = nc.dram_tensor("w1", [ELOC, D, H], BF16, kind="ExternalInput")
    b1_in = nc.dram_tensor("b1", [ELOC, H], FP32, kind="ExternalInput")
    w2_in = nc.dram_tensor("w2", [ELOC, H, C], BF16, kind="ExternalInput")
    b2_in = nc.dram_tensor("b2", [ELOC, P, C], FP32, kind="ExternalInput")
    sel_in = nc.dram_tensor("sel", [ELOC, P, E], FP32, kind="ExternalInput")
    if combine == "rs":
        out_t = nc.dram_tensor("out", [B // N_CORES, C], FP32, kind="ExternalOutput")
    else:
        out_t = nc.dram_tensor("out", [B, C], FP32, kind="ExternalOutput")

    with tile.TileContext(nc) as tc:
        if repeat == 0:
            # minimal body for differential timing: zero-fill out_t only
            with tc.tile_pool(name="z", bufs=1) as zp:
                zt = zp.tile([P, C], FP32)
                nc.vector.memset(zt[:], 0.0)
                for i in range(out_t.shape[0] // P):
                    nc.sync.dma_start(out_t[i * P:(i + 1) * P, :], zt[:])
        for _ in range(repeat):
            _emit_body(
                nc, tc, x_in, wg_in, bg_in, w1_in, b1_in, w2_in, b2_in, sel_in,
                out_t, combine, noind,
            )
    if not for_sim:
        _split_excess_waits(nc)
    return nc


def _emit_body(nc, tc, x_in, wg_in, bg_in, w1_in, b1_in, w2_in, b2_in, sel_in,
               out_t, combine, noind=False):
    from contextlib import ExitStack

    with ExitStack() as ctx:
        dram = ctx.enter_context(tc.tile_pool(name="dram", bufs=1, space="DRAM"))
        consts = ctx.enter_context(tc.tile_pool(name="consts", bufs=1))
        xrow_p = ctx.enter_context(tc.tile_pool(name="xrow", bufs=3))
        xt_p = ctx.enter_context(tc.tile_pool(name="xt", bufs=3))
        gate_p = ctx.enter_context(tc.tile_pool(name="gate", bufs=4))
        route_p = ctx.enter_context(tc.tile_pool(name="route", bufs=1))
        wres_p = ctx.enter_context(tc.tile_pool(name="wres", bufs=1))
        ht_p = ctx.enter_context(tc.tile_pool(name="ht", bufs=1))
        xdt_p = ctx.enter_context(tc.tile_pool(name="xdt", bufs=1))
        ffn_p = ctx.enter_context(tc.tile_pool(name="ffn", bufs=2))
        psum = ctx.enter_context(tc.tile_pool(name="psum", bufs=2, space="PSUM"))
        psg = ctx.enter_context(tc.tile_pool(name="psg", bufs=1, space="PSUM"))

        # ------------------------------------------------ DRAM scratch
        idlist = dram.tile([NSLOT, 4], FP32)
        partial = dram.tile([B + P, C], FP32)
        if combine == "rs":
            rsout = dram.tile([B // N_CORES, C], FP32)

        # ------------------------------------------------ constants
        ident = consts.tile([P, P], FP32)
        make_identity(nc, ident[:])

        # strict upper-triangular ones (TRI[p, c] = 1 iff c > p) for the
        # cross-partition exclusive prefix sum
        tri_i = consts.tile([P, P], INT32)
        nc.gpsimd.iota(tri_i[:], pattern=[[1, P]], base=0, channel_multiplier=-1)
        tri = consts.tile([P, P], FP32)
        nc.vector.tensor_scalar(tri[:], tri_i[:], 0, None, op0=mybir.AluOpType.is_gt)

        wgsb = consts.tile([P, DC, E], FP32)
        nc.sync.dma_start(wgsb[:], wg_in[:].rearrange("(c p) e -> p c e", p=P))
        bgbc = consts.tile([P, E], FP32)
        nc.sync.dma_start(bgbc[:], bg_in[:])
        selbc = consts.tile([P, ELOC, E], FP32)
        nc.sync.dma_start(selbc[:], sel_in[:].rearrange("j p e -> p j e"))
        b1sb = consts.tile([P, ELOC, HC], FP32)
        nc.sync.dma_start(
            b1sb[:], b1_in[:].rearrange("j (m p) -> p j m", p=P)
        )
        b2bc = consts.tile([P, ELOC, C], FP32)
        nc.sync.dma_start(b2bc[:], b2_in[:].rearrange("j p c -> p j c"))

        # token ids, f32 (exact for < 2^24)
        tokid_i = consts.tile([P, NT], INT32)
        nc.gpsimd.iota(tokid_i[:], pattern=[[P, NT]], base=0, channel_multiplier=1)
        tokf = consts.tile([P, NT], FP32)
        nc.vector.tensor_copy(tokf[:], tokid_i[:])

        # ------------------------------------------------ zero-init partial
        zsb = consts.tile([P, C], FP32)
        nc.vector.memset(zsb[:], 0.0)
        for i in range((B + P) // P):
            nc.sync.dma_start(partial[i * P:(i + 1) * P, :], zsb[:])

        # prefill idlist with (id=TRASH, wv=0, wv=0, 0) for padding slots
        pref = consts.tile([P, NSLOT // P, 4], FP32)
        nc.vector.memset(pref[:], 0.0)
        nc.vector.memset(pref[:, :, 3:4], float(TRASH))
        nc.sync.dma_start(
            idlist[:].rearrange("(a p) m -> p a m", p=P), pref[:]
        )

        # ------------------------------------------------ routing buffers
        g0buf = route_p.tile([P, NT], FP32)    # gate prob of local expert 0
        g1buf = route_p.tile([P, NT], FP32)
        mU = [route_p.tile([P, NT], FP32, name=f"mU{j}") for j in range(ELOC)]
        m1b = [route_p.tile([P, NT], FP32, name=f"m1b{j}") for j in range(ELOC)]

        gbufs = [g0buf, g1buf]

        # ================================================ phase A: gate+masks
        # gate logits for ALL tokens land in glogA [P, NT, E]; the softmax /
        # top-2 / local-expert math then runs as a handful of big batched DVE
        # ops instead of ~12 tiny ops per token tile.
        glogA = gate_p.tile([P, NT, E], FP32, tag="glogA")
        FG = 4                        # f-tiles per PSUM gate tile
        for fg in range(NT // FG):
            gps = psg.tile([P, FG, E], FP32, tag="gps")
            for f4 in range(FG):
                f = fg * FG + f4
                xrow = xrow_p.tile([P, D], FP32, tag="xrow")
                nc.sync.dma_start(xrow[:], x_in[f * P:(f + 1) * P, :])

                xt = xt_p.tile([P, DC, P], FP32, tag="xt")
                for g in range(DC // 4):
                    pst = psum.tile([P, 4, P], FP32, tag="pst")
                    for c4 in range(4):
                        c = g * 4 + c4
                        nc.tensor.transpose(
                            pst[:, c4, :], xrow[:, c * P:(c + 1) * P], ident[:]
                        )
                    nc.vector.tensor_copy(xt[:, g * 4:(g + 1) * 4, :], pst[:])

                for c in range(DC):
                    nc.tensor.matmul(
                        gps[:, f4, :], lhsT=xt[:, c, :], rhs=wgsb[:, c, :],
                        start=(c == 0), stop=(c == DC - 1),
                    )
            nc.scalar.copy(glogA[:, fg * FG:(fg + 1) * FG, :], gps[:])

        def bc_last(ap2d):   # [P, NT] -> [P, NT, E] with stride-0 inner
            return ap2d.unsqueeze(2).broadcast_to([P, NT, E])

        def bc_mid(ap2d):    # [P, E] -> [P, NT, E] with stride-0 middle
            return ap2d.unsqueeze(1).broadcast_to([P, NT, E])

        TT = mybir.AluOpType
        nc.vector.tensor_tensor(
            out=glogA[:], in0=glogA[:], in1=bc_mid(bgbc[:]), op=TT.add)
        m1 = route_p.tile([P, NT], FP32)         # max logit per token
        nc.vector.reduce_max(m1[:], glogA[:], axis=mybir.AxisListType.X)
        expsA = gate_p.tile([P, NT, E], FP32, tag="expsA")
        nc.vector.tensor_tensor(
            out=expsA[:], in0=glogA[:], in1=bc_last(m1[:]), op=TT.subtract)
        nc.scalar.activation(
            expsA[:], expsA[:], mybir.ActivationFunctionType.Exp)
        seA = route_p.tile([P, NT], FP32)
        nc.vector.reduce_sum(seA[:], expsA[:], axis=mybir.AxisListType.X)
        rsA = route_p.tile([P, NT], FP32)
        nc.vector.reciprocal(rsA[:], seA[:])
        gateA = gate_p.tile([P, NT, E], FP32, tag="gateA")
        nc.vector.tensor_tensor(
            out=gateA[:], in0=expsA[:], in1=bc_last(rsA[:]), op=TT.mult)

        # top-1 / top-2 gate probs per token
        g1m = route_p.tile([P, NT], FP32)
        nc.vector.reduce_max(g1m[:], gateA[:], axis=mybir.AxisListType.X)
        msk = gate_p.tile([P, NT, E], FP32, tag="msk")
        nc.vector.tensor_tensor(
            out=msk[:], in0=gateA[:], in1=bc_last(g1m[:]), op=TT.is_ge)
        nc.vector.tensor_tensor(out=msk[:], in0=msk[:], in1=gateA[:], op=TT.mult)
        nc.vector.tensor_tensor(out=msk[:], in0=gateA[:], in1=msk[:], op=TT.subtract)
        g2m = route_p.tile([P, NT], FP32)
        nc.vector.reduce_max(g2m[:], msk[:], axis=mybir.AxisListType.X)

        # local-expert gate prob + top-1/top-2 membership masks
        scrA = gate_p.tile([P, NT, E], FP32, tag="msk")
        for j in range(ELOC):
            nc.vector.tensor_tensor(
                out=scrA[:], in0=gateA[:], in1=bc_mid(selbc[:, j, :]), op=TT.mult)
            nc.vector.reduce_sum(
                gbufs[j][:], scrA[:], axis=mybir.AxisListType.X)
            nc.vector.tensor_tensor(
                out=mU[j][:], in0=gbufs[j][:], in1=g2m[:], op=TT.is_ge)
            nc.vector.tensor_tensor(
                out=m1b[j][:], in0=gbufs[j][:], in1=g1m[:], op=TT.is_ge)

        # ================================================ phase B: compaction
        # per-partition inclusive prefix along f, then exclusive via subtract
        rank = []
        tot = route_p.tile([P, ELOC], FP32)
        for j in range(ELOC):
            inc = route_p.tile([P, NT], FP32, tag=f"inc{j}")
            nc.vector.tensor_tensor_scan(
                out=inc[:], data0=mU[j][:], data1=mU[j][:], initial=0.0,
                op0=mybir.AluOpType.add, op1=mybir.AluOpType.bypass,
            )
            exc = route_p.tile([P, NT], FP32, tag=f"exc{j}")
            nc.vector.tensor_sub(exc[:], inc[:], mU[j][:])
            nc.vector.tensor_copy(tot[:, j:j + 1], inc[:, NT - 1:NT])
            rank.append(exc)

        offp = psg.tile([P, ELOC], FP32, tag="offp")
        nc.tensor.matmul(offp[:], lhsT=tri[:], rhs=tot[:], start=True, stop=True)
        offs = route_p.tile([P, ELOC], FP32)
        nc.vector.tensor_copy(offs[:], offp[:])

        for j in range(ELOC):
            nc.vector.tensor_scalar(
                rank[j][:], rank[j][:], offs[:, j:j + 1], float(j * CAP),
                op0=mybir.AluOpType.add, op1=mybir.AluOpType.add,
            )

        # slots per choice; non-local tokens get OOB
        slots_i = []
        for choice in range(2):
            own = route_p.tile([P, NT], FP32, tag=f"own{choice}")
            slot = route_p.tile([P, NT], FP32, tag=f"slot{choice}")
            nc.vector.memset(slot[:], 0.0)
            nc.vector.memset(own[:], 0.0)
            for j in range(ELOC):
                cm = route_p.tile([P, NT], FP32, tag="cm")
                if choice == 0:
                    nc.vector.tensor_copy(cm[:], m1b[j][:])
                else:
                    nc.vector.tensor_sub(cm[:], mU[j][:], m1b[j][:])
                nc.vector.tensor_add(own[:], own[:], cm[:])
                nc.vector.tensor_mul(cm[:], cm[:], rank[j][:])
                nc.vector.tensor_add(slot[:], slot[:], cm[:])
            # slot += (1 - own) * OOB
            nc.vector.tensor_scalar(
                own[:], own[:], -float(OOB), float(OOB),
                op0=mybir.AluOpType.mult, op1=mybir.AluOpType.add,
            )
            nc.vector.tensor_add(slot[:], slot[:], own[:])
            si = route_p.tile([P, NT], INT32, tag=f"sloti{choice}")
            nc.vector.tensor_copy(si[:], slot[:])
            slots_i.append(si)

        # metadata rows (tokid, gsel0, gsel1, 0) scattered into idlist
        meta = route_p.tile([P, NT, 4], FP32)
        nc.vector.tensor_copy(meta[:, :, 0], tokf[:])
        nc.vector.tensor_copy(meta[:, :, 1], g0buf[:])
        nc.vector.tensor_copy(meta[:, :, 2], g1buf[:])
        nc.vector.tensor_copy(meta[:, :, 3], tokf[:])
        # HW only honors [128, 1] offset vectors (multi-column offsets are a
        # sim-only behavior), so emit one small scatter per (choice, token tile)
        if noind:
            # timing probe: same byte volume, dense HWDGE writes instead of
            # SWDGE scatters (results are wrong — timing only)
            for choice in range(2):
                for f in range(NT):
                    nc.sync.dma_start(
                        idlist[(f % (NSLOT // P)) * P:(f % (NSLOT // P)) * P + P, :],
                        meta[:, f, :],
                    )
        else:
            bc_reg = nc.gpsimd.to_reg(NSLOT - 1)
            for choice in range(2):
                for f in range(NT):
                    nc.gpsimd.indirect_dma_start(
                        out=idlist[:],
                        out_offset=bass.IndirectOffsetOnAxis(
                            ap=slots_i[choice][:, f:f + 1], axis=0
                        ),
                        in_=meta[:, f, :],
                        in_offset=None,
                        bounds_check=bc_reg,
                        oob_is_err=False,
                    )

        # read the compacted list back
        idsb = route_p.tile([P, NSLOT // P, 4], FP32)
        nc.sync.dma_start(idsb[:], idlist[:].rearrange("(a p) m -> p a m", p=P))
        idcast = route_p.tile([P, NSLOT // P], INT32)
        nc.vector.tensor_copy(idcast[:], idsb[:, :, 0])
        idscat = route_p.tile([P, NSLOT // P], INT32)
        nc.vector.tensor_copy(idscat[:], idsb[:, :, 3])

        # ================================================ phase C: expert FFN
        for j in range(ELOC):
            # gather this expert's x rows tile by tile: xg[p, :] = x[id, :],
            # then transpose -> xdt[:, c, slot]
            xdt = xdt_p.tile([P, DC, CAP], BF16, tag="xdt")
            for a in range(CAPT):
                xg = ffn_p.tile([P, D], FP32, tag="xg")
                if noind:
                    nc.sync.dma_start(xg[:], x_in[a * P:(a + 1) * P, :])
                else:
                    nc.gpsimd.indirect_dma_start(
                        out=xg[:],
                        out_offset=None,
                        in_=x_in[:],
                        in_offset=bass.IndirectOffsetOnAxis(
                            ap=idcast[:, j * CAPT + a:j * CAPT + a + 1], axis=0
                        ),
                    )
                for g in range(DC // 4):
                    pst = psum.tile([P, 4, P], FP32, tag="pst")
                    for c4 in range(4):
                        c = g * 4 + c4
                        nc.tensor.transpose(
                            pst[:, c4, :], xg[:, c * P:(c + 1) * P], ident[:]
                        )
                    nc.vector.tensor_copy(
                        xdt[:, g * 4:(g + 1) * 4, a * P:(a + 1) * P], pst[:]
                    )

            # fc1: hT[m-tile] = sum_c W1[c,m]^T @ xdt[c]  (+ b1)
            w1r = wres_p.tile([P, DC, H], BF16, tag="wres1")
            nc.sync.dma_start(
                w1r[:], w1_in[j].rearrange("(c p) h -> p c h", p=P)
            )
            hts = ht_p.tile([P, HC, CAP], BF16, tag="hts")
            segs = [(0, 512), (512, CAP)]
            for m in range(HC):
                for s0, s1 in segs:
                    hp = psum.tile([P, 512], FP32, tag="hp")
                    for c in range(DC):
                        nc.tensor.matmul(
                            hp[:, :s1 - s0],
                            lhsT=w1r[:, c, m * P:(m + 1) * P],
                            rhs=xdt[:, c, s0:s1],
                            start=(c == 0), stop=(c == DC - 1),
                        )
                    nc.vector.tensor_scalar(
                        hts[:, m, s0:s1], hp[:, :s1 - s0],
                        b1sb[:, j, m:m + 1], None, op0=mybir.AluOpType.add,
                    )

            # fc2 + log_softmax + weight + scatter-add
            w2r = wres_p.tile([P, HC, C], BF16, tag="wres2")
            nc.sync.dma_start(
                w2r[:], w2_in[j].rearrange("(kc p) cc -> p kc cc", p=P)
            )
            csegs = [(0, 512), (512, C)]
            for a in range(CAPT):
                lps = []
                for s0, s1 in csegs:
                    lp = psum.tile([P, 512], FP32, tag="lp")
                    for kc in range(HC):
                        nc.tensor.matmul(
                            lp[:, :s1 - s0],
                            lhsT=hts[:, kc, a * P:(a + 1) * P],
                            rhs=w2r[:, kc, s0:s1],
                            start=(kc == 0), stop=(kc == HC - 1),
                        )
                    lps.append(lp)
                pst_sb = ffn_p.tile([P, C], FP32, tag="logits")
                for (s0, s1), lp in zip(csegs, lps):
                    nc.vector.tensor_add(
                        pst_sb[:, s0:s1], lp[:, :s1 - s0], b2bc[:, j, s0:s1]
                    )
                mx = ffn_p.tile([P, 1], FP32, tag="mx")
                nc.vector.reduce_max(mx[:], pst_sb[:], axis=mybir.AxisListType.X)
                nmx = ffn_p.tile([P, 1], FP32, tag="nmx")
                nc.vector.tensor_scalar_mul(nmx[:], mx[:], -1.0)
                expv = ffn_p.tile([P, C], FP32, tag="expv")
                sev = ffn_p.tile([P, 1], FP32, tag="sev")
                nc.scalar.activation(
                    expv[:], pst_sb[:], mybir.ActivationFunctionType.Exp,
                    bias=nmx[:], scale=1.0, accum_out=sev[:],
                )
                lnz = ffn_p.tile([P, 1], FP32, tag="lnz")
                nc.scalar.activation(
                    lnz[:], sev[:], mybir.ActivationFunctionType.Ln,
                )
                total = ffn_p.tile([P, 1], FP32, tag="total")
                nc.vector.tensor_add(total[:], mx[:], lnz[:])
                outsb = ffn_p.tile([P, C], FP32, tag="outsb")
                nc.vector.tensor_scalar(
                    outsb[:], pst_sb[:], total[:],
                    idsb[:, j * CAPT + a, 1 + j:2 + j],
                    op0=mybir.AluOpType.subtract, op1=mybir.AluOpType.mult,
                )
                # expert 0 scatters onto freshly zeroed rows: plain writes
                # (pad rows all write zeros to the trash row - benign); expert 1
                # must accumulate (a token can route to both local experts).
                if noind:
                    nc.sync.dma_start(partial[a * P:(a + 1) * P, :], outsb[:])
                else:
                    nc.gpsimd.indirect_dma_start(
                        out=partial[:],
                        out_offset=bass.IndirectOffsetOnAxis(
                            ap=idscat[:, j * CAPT + a:j * CAPT + a + 1], axis=0
                        ),
                        in_=outsb[:],
                        in_offset=None,
                        compute_op=(mybir.AluOpType.bypass if j == 0
                                    else mybir.AluOpType.add),
                    )

        # ================================================ phase D: combine
        if combine == "rs":
            nc.gpsimd.collective_compute(
                "ReduceScatter",
                mybir.AluOpType.add,
                replica_groups=[list(range(N_CORES))],
                ins=[partial[:B, :].opt()],
                outs=[rsout[:].opt()],
            )
            nc.sync.dma_start(out_t[:], rsout[:])
        else:
            ot = ffn_p.tile([P, C], FP32, tag="otile")
            for i in range(B // P):
                nc.sync.dma_start(ot[:], partial[i * P:(i + 1) * P, :])
                nc.sync.dma_start(out_t[i * P:(i + 1) * P, :], ot[:])


# ---------------------------------------------------------------- host glue
_CACHE = {}


def _get_nc(combine: str):
    if combine not in _CACHE:
        _CACHE[combine] = build_bass(combine)
    return _CACHE[combine]


def make_in_maps(x, Wg, bg, W1, b1, W2, b2):
    import ml_dtypes
    bf16 = np.dtype(ml_dtypes.bfloat16)
    x = np.ascontiguousarray(np.asarray(x, np.float32))
    Wg = np.ascontiguousarray(np.asarray(Wg, np.float32))
    bg = np.tile(np.asarray(bg, np.float32).reshape(1, E), (P, 1))
    W1 = np.ascontiguousarray(np.asarray(W1, np.float32).astype(bf16))
    b1 = np.ascontiguousarray(np.asarray(b1, np.float32))
    W2 = np.ascontiguousarray(np.asarray(W2, np.float32).astype(bf16))
    b2 = np.ascontiguousarray(np.asarray(b2, np.float32))
    maps = []
    for i in range(N_CORES):
        lo = i * ELOC
        sel = np.zeros((ELOC, 1, E), np.float32)
        for j in range(ELOC):
            sel[j, 0, lo + j] = 1.0
        # replicated across partitions host-side (partition_broadcast is
        # unsupported by this walrus build)
        maps.append({
            "x": x,
            "wg": Wg,
            "bg": bg,
            "w1": np.ascontiguousarray(W1[lo:lo + ELOC]),
            "b1": np.ascontiguousarray(b1[lo:lo + ELOC]),
            "w2": np.ascontiguousarray(W2[lo:lo + ELOC]),
            "b2": np.ascontiguousarray(
                np.tile(b2[lo:lo + ELOC].reshape(ELOC, 1, C), (1, P, 1))),
            "sel": np.ascontiguousarray(np.tile(sel, (1, P, 1))),
        })
    return maps


def _assert_capacity(x, Wg, bg):
    gate = np.asarray(x, np.float32) @ np.asarray(Wg, np.float32)
    gate += np.asarray(bg, np.float32).reshape(1, E)
    order = np.argsort(-gate, axis=1)[:, :TOPK]
    counts = np.bincount(order.ravel(), minlength=E)
    assert counts.max() <= CAP, (
        f"per-expert token load {counts.max()} exceeds CAP={CAP}; "
        f"raise CAP in kernel.py"
    )


def kernel(x, Wg, bg, W1, b1, W2, b2, k):
    assert int(k) == TOPK
    _assert_capacity(x, Wg, bg)
    combine = _COMBINE
    nc = _get_nc(combine)
    maps = make_in_maps(x, Wg, bg, W1, b1, W2, b2)
    res = run_bass_kernel_spmd(nc, maps, core_ids=list(range(N_CORES)))
    if combine == "rs":
        out = np.concatenate([res.results[i]["out"] for i in range(N_CORES)], axis=0)
    else:
        out = np.sum([res.results[i]["out"] for i in range(N_CORES)], axis=0)
    return out.astype(np.float32)


if __name__ == "__main__":
    rng = np.random.default_rng(0)
    x = rng.standard_normal((B, D), np.float32)
    Wg = rng.standard_normal((D, E), np.float32) / np.sqrt(D)
    bg = np.zeros((E,), np.float32)
    W1 = (rng.standard_normal((E, D, H)) / np.sqrt(D)).astype(np.float32)
    b1 = np.zeros((E, H), np.float32)
    W2 = (rng.standard_normal((E, H, C)) / np.sqrt(H)).astype(np.float32)
    b2 = np.zeros((E, C), np.float32)
    out = kernel(x, Wg, bg, W1, b1, W2, b2, 2)
    print("kernel ran, out:", out.shape, out.dtype, float(np.abs(out).max()))

